# revision 5
# baseline (speedup 1.0000x reference)
"""GIN GNN kernel for 8 TRN2 NeuronCores — batched-gather S-matmul design (v2).

Key idea vs v1: the v1 kernel issued one gpsimd indirect-DMA per 128 edges
(~1us fixed SWDGE overhead each, 1914 instructions = 2.3ms serialized on
GpSimd). v2 gathers thousands of edge rows per instruction with dma_gather
(994ns + 0.34ns/row), cutting GpSimd busy time ~10x.

Structure (per core, SPMD-identical program, per-core data in tensors):
- Host precomputes y0 = x @ W1_1 (linearity: aggregating y0 == aggregating x
  then multiplying by W1_1), so all 3 layers share one pipeline: gather y rows
  per edge, one-hot S-matmul aggregation into PSUM [H, 256] windows, BN+ReLU
  via scalar activation (per-partition scale/bias), W2 matmul, ReLU, then
  y_{l+1} production via lhsT-swap matmuls (node-major).
- Edges sorted by (dst window of 256 slots, src core, dst slot); per
  (window, src) group padded to 128-edge columns. Per src core, columns are
  gathered from that core's section of the (replicated/AllGathered) y table
  in a few thousand-row dma_gather calls.
- Aggregation: per window w, one 3D is_equal builds all its one-hot S
  columns; one matmul per 128-edge column accumulates into PSUM; self term
  added via identity matmuls from a direct DMA of the window's own y rows.
- Layers 1->2 and 2->3 need an AllGather of the per-core y table (edges cross
  cores since edge_index is random).
- Layer 3 tail: node-major W2 via lhsT swap, pooling via per-graph-block
  one-hot matmuls into PSUM accumulators, FC head per 128-graph block
  (same as v1).
"""

import sys

sys.path.insert(0, "/opt/trn_rl_repo")

import numpy as np
import concourse.bass as bass
import concourse.bacc as bacc
import concourse.bass_isa as bass_isa
import concourse.mybir as mybir
import concourse.tile as tile
from concourse import bass_utils
from concourse.masks import make_identity

P = 128
W = 256           # PSUM aggregation window (dst slots)
BN_EPS = 1e-5
NPF = np.float16  # host dtype matching mybir.dt.float16


class Cfg:
    def __init__(self, n_nodes, n_edges, n_graphs, f_node, h, ncores, gwin=16,
                 maxc=6):
        self.N, self.E, self.G, self.F, self.H = n_nodes, n_edges, n_graphs, f_node, h
        self.NCORES = ncores
        self.G_PER_CORE = n_graphs // ncores
        self.GWIN = gwin  # windows per gather group
        self.MAXC = maxc  # max 128-row columns per dma_gather call (ring cap)


def preprocess_graph(cfg, edge_index, batch):
    """Partition nodes by graph quantiles; organize edges into
    (dst-window, src-core) column groups shared across layers."""
    nco, gpc = cfg.NCORES, cfg.G_PER_CORE
    batch = np.asarray(batch)
    ei = np.asarray(edge_index)
    src, dst = ei[0].astype(np.int64), ei[1].astype(np.int64)

    node_start = np.searchsorted(batch, np.arange(nco + 1) * gpc, side="left")
    counts = np.diff(node_start)
    PAD_N = int(np.ceil(counts.max() / W) * W)
    assert PAD_N < 32768, PAD_N  # int16 gather index range
    NW = PAD_N // W
    NT = PAD_N // P

    core_of = np.searchsorted(node_start, np.arange(cfg.N), side="right") - 1
    local = np.arange(cfg.N) - node_start[core_of]

    dcore = core_of[dst]
    dloc = local[dst]
    dwin = dloc // W
    dslot = dloc % W
    score = core_of[src]
    sloc = local[src]

    # per-core, per-(window, src-core) counts
    cnt = np.zeros((nco, NW, nco), np.int64)
    np.add.at(cnt, (dcore, dwin, score), 1)
    ncols = np.maximum(np.ceil(cnt / P).astype(np.int64).max(axis=0), 1)  # [NW, nco]

    # column layout: per src-core stream (window-major); global edst layout
    # (window-major, src-major inside window)
    colbase_s = np.zeros((nco, NW + 1), np.int64)  # per src: col offset per window
    for s in range(nco):
        colbase_s[s, 1:] = np.cumsum(ncols[:, s])
    total_cols_s = colbase_s[:, -1]              # [nco]
    slotbase_s = np.concatenate([[0], np.cumsum(total_cols_s * P)])
    SLOTS_TOTAL = int(slotbase_s[-1])

    # global edst column index for (w, s, j)
    ncw = ncols.sum(axis=1)                      # cols per window
    cw0 = np.concatenate([[0], np.cumsum(ncw)])  # window col offsets
    TOTC = int(cw0[-1])
    NCW_MAX = int(ncw.max())

    # window_cols[w] = list of (s, col_in_s_stream)
    window_cols = []
    for w in range(NW):
        cols = []
        for s in range(nco):
            for j in range(ncols[w, s]):
                cols.append((s, int(colbase_s[s, w] + j)))
        window_cols.append(cols)

    # gather groups: per src-core, window ranges of GWIN
    GWIN = cfg.GWIN
    NG = int(np.ceil(NW / GWIN))
    gcalls = []  # (s, C0, C1) column range in s's stream
    NCAL_MAX = np.zeros(nco, np.int64)
    for g in range(NG):
        w0, w1 = g * GWIN, min((g + 1) * GWIN, NW)
        for s in range(nco):
            C0, C1 = int(colbase_s[s, w0]), int(colbase_s[s, w1])
            gcalls.append((g, s, C0, C1))
            NCAL_MAX[s] = max(NCAL_MAX[s], C1 - C0)

    # fill per-core eidx / edst
    eidx = np.zeros((nco, SLOTS_TOTAL), np.int16)
    edst = np.full((nco, P, TOTC), 3000.0, np.float16)
    order = np.lexsort((dslot, score, dwin, dcore))
    k_s, w_s, s_s = dcore[order], dwin[order], score[order]
    sl_s, slot_s = sloc[order], dslot[order]
    grp = (k_s * NW + w_s) * nco + s_s
    grp_change = np.concatenate([[True], grp[1:] != grp[:-1]])
    grp_first = np.where(grp_change)[0]
    grp_id = np.cumsum(grp_change) - 1
    pos = np.arange(len(order)) - grp_first[grp_id]   # pos within (k,w,s) group
    col_in_grp = pos // P
    row = pos % P
    slotpos = slotbase_s[s_s] + (colbase_s[s_s, w_s] + col_in_grp) * P + row
    eidx[k_s, slotpos] = sl_s.astype(np.int16)
    gcol = cw0[w_s] + (colbase_s[s_s, w_s] - colbase_s[np.zeros_like(s_s), w_s] * 0)
    # global edst col: cw0[w] + (cols of src cores < s in window w) + col_in_grp
    pre_s = np.zeros((NW, nco), np.int64)
    pre_s[:, 1:] = np.cumsum(ncols[:, :-1], axis=1)
    gcol = cw0[w_s] + pre_s[w_s, s_s] + col_in_grp
    edst[k_s, row, gcol] = slot_s

    # wrap eidx: pos i -> [i%16, i//16], replicate to 128 partitions
    eidx_w = np.empty((nco, 128, SLOTS_TOTAL // 16), np.int16)
    for k in range(nco):
        eidx_w[k] = np.tile(eidx[k].reshape(-1, 16).T, (8, 1))

    # batchT + pooling ranges (natural node order)
    batchT = np.full((nco, P, NT), -1.0, np.float16)
    JB = int(np.ceil(gpc / P))
    TJ0 = np.full(JB, NT, np.int64)
    TJ1 = np.zeros(JB, np.int64)
    for k in range(nco):
        bl = batch[node_start[k]:node_start[k + 1]] - k * gpc
        bt = np.full(PAD_N, -1.0, np.float32)
        bt[:counts[k]] = bl
        batchT[k] = bt.reshape(NT, P).T
        for J in range(JB):
            lo = np.searchsorted(bl, J * P, side="left")
            hi = np.searchsorted(bl, min((J + 1) * P, gpc), side="left")
            if hi > lo:
                TJ0[J] = min(TJ0[J], lo // P)
                TJ1[J] = max(TJ1[J], (hi - 1) // P + 1)
    TJ0 = np.minimum(TJ0, TJ1)

    meta = dict(PAD_N=PAD_N, NW=NW, NT=NT, SLOTS_TOTAL=SLOTS_TOTAL, TOTC=TOTC,
                NCW_MAX=NCW_MAX, ncols=ncols, window_cols=window_cols,
                gcalls=gcalls, NCAL_MAX=NCAL_MAX.tolist(), colbase_s=colbase_s,
                slotbase_s=slotbase_s, cw0=cw0, NG=NG,
                JB=JB, TJ0=TJ0.tolist(), TJ1=TJ1.tolist(),
                node_start=node_start, counts=counts)
    percore = dict(eidx=eidx_w, edst=edst, batchT=batchT)
    return meta, percore


def _fix_gather_queues(nc):
    """Rewrite dma_gather queue_num to match the tile sem-lane rotation.

    Tile assigns the 8 SWDGE semaphore lanes round-robin over Pool-engine DMA
    instructions in final scheduled order; the runtime locks each sem lane to
    one SWDGE queue. Setting queue = lane % 4 post-scheduling keeps the
    mapping consistent regardless of how the scheduler ordered the gathers.
    """
    ctr = 0
    for f in nc.m.functions:
        for bb in f.blocks:
            for ins in bb.instructions:
                if (isinstance(ins, bass_isa.AnyDMAInstruction)
                        and ins.engine == mybir.EngineType.Pool
                        and not isinstance(
                            ins, getattr(bass_isa, "UserSyncedRemoteDMADescs",
                                         ()))):
                    if isinstance(ins, mybir.InstDMAGatherAnt):
                        ins.queue_num = (ctr % 8) % 4
                    ctr += 1


def fold_bn(w1b, gamma, beta, rmean, rvar):
    s = gamma / np.sqrt(rvar + BN_EPS)
    t = (w1b - rmean) * s + beta
    return s.astype(np.float32), t.astype(np.float32)


def build(cfg, meta):
    H = cfg.H
    nco = cfg.NCORES
    PAD_N, NW, NT = meta["PAD_N"], meta["NW"], meta["NT"]
    SLOTS_TOTAL, TOTC, NCW_MAX = meta["SLOTS_TOTAL"], meta["TOTC"], meta["NCW_MAX"]
    window_cols, gcalls = meta["window_cols"], meta["gcalls"]
    colbase_s, slotbase_s = meta["colbase_s"], meta["slotbase_s"]
    NCAL_MAX = meta["NCAL_MAX"]
    NG, GWIN = meta["NG"], cfg.GWIN
    JB, TJ0, TJ1 = meta["JB"], meta["TJ0"], meta["TJ1"]
    f32 = mybir.dt.float32
    bf = mybir.dt.float16

    nc = bacc.Bacc("TRN2", target_bir_lowering=False, debug=False, num_devices=nco,
                   enable_asserts=False, num_swdge_queues=4)
    tc = tile.TileContext(nc, num_cores=nco)

    def dram_in(name, shape, dt=f32):
        return nc.dram_tensor(name, shape, dt, kind="ExternalInput").ap()

    y0g = dram_in("y0g", [nco * PAD_N, H], bf)      # replicated y0 = x@W1_1
    y0l = dram_in("y0l", [PAD_N, H], bf)            # own slice (self term)
    eidx = dram_in("eidx", [P, SLOTS_TOTAL // 16], mybir.dt.int16)
    edst = dram_in("edst", [P, TOTC], bf)
    batchT = dram_in("batchT", [P, NT], bf)
    w2 = {l: dram_in(f"w2_{l}", [H, H], bf) for l in (1, 2, 3)}
    w1n = {l: dram_in(f"w1n_{l}", [H, H], bf) for l in (2, 3)}
    bn_s = {l: dram_in(f"bn_s_{l}", [H, 1]) for l in (1, 2, 3)}
    bn_t = {l: dram_in(f"bn_t_{l}", [H, 1]) for l in (1, 2, 3)}
    b2 = {l: dram_in(f"b2_{l}", [H, 1]) for l in (1, 2)}
    b2row3 = dram_in("b2row3", [1, H], bf)
    wfc1 = dram_in("wfc1", [H, H // 2], bf)
    bfc1 = dram_in("bfc1", [H // 2, 1])
    wfc2 = dram_in("wfc2", [H // 2, 1], bf)
    bfc2 = dram_in("bfc2", [1, 1])

    out = nc.dram_tensor("out", [1, JB * P], f32, kind="ExternalOutput").ap()

    RELU = mybir.ActivationFunctionType.Relu
    IDENT = mybir.ActivationFunctionType.Identity

    with tc:
        with (
            tc.tile_pool(name="const", bufs=1) as cpool,
            tc.tile_pool(name="gat", bufs=2) as gpool,
            tc.tile_pool(name="smat", bufs=3) as spool,
            tc.tile_pool(name="self", bufs=3) as sfpool,
            tc.tile_pool(name="work", bufs=3) as wpool,
            tc.tile_pool(name="yout", bufs=3) as ypool,
            tc.tile_pool(name="zps", bufs=2, space="PSUM") as zpool,
            tc.tile_pool(name="mmps", bufs=3, space="PSUM") as mmpool,
            tc.tile_pool(name="pool_ps", bufs=3, space="PSUM") as ppool,
            tc.tile_pool(name="dram", bufs=1, space="DRAM") as dpool,
        ):
            # ---- constants ----
            iota_i = cpool.tile([P, W], mybir.dt.int32)
            nc.gpsimd.iota(iota_i[:], pattern=[[1, W]], base=0, channel_multiplier=0)
            iota_f = cpool.tile([P, W], f32)
            nc.vector.tensor_copy(iota_f[:], iota_i[:])
            iota_h = cpool.tile([P, W], bf)
            nc.vector.tensor_copy(iota_h[:], iota_i[:])
            ident = cpool.tile([P, P], bf)
            make_identity(nc, ident[:])
            ones_row = cpool.tile([1, P], bf)
            nc.vector.memset(ones_row[:], 1.0)

            eidx_sb = cpool.tile([P, SLOTS_TOTAL // 16], mybir.dt.int16)
            nc.sync.dma_start(eidx_sb[:], eidx[:, :])
            edst_sb = cpool.tile([P, TOTC], bf)
            nc.sync.dma_start(edst_sb[:], edst[:, :])
            batch_sb = cpool.tile([P, NT], bf)
            nc.sync.dma_start(batch_sb[:], batchT[:, :])

            w2_sb, w1n_sb, bns_sb, bnt_sb, b2_sb = {}, {}, {}, {}, {}
            for l in (1, 2, 3):
                w2_sb[l] = cpool.tile([H, H], bf, tag=f"w2_{l}", name=f"w2sb_{l}")
                nc.sync.dma_start(w2_sb[l][:], w2[l][:, :])
                bns_sb[l] = cpool.tile([H, 1], f32, tag=f"bns_{l}", name=f"bnssb_{l}")
                nc.sync.dma_start(bns_sb[l][:], bn_s[l][:, :])
                bnt_sb[l] = cpool.tile([H, 1], f32, tag=f"bnt_{l}", name=f"bntsb_{l}")
                nc.sync.dma_start(bnt_sb[l][:], bn_t[l][:, :])
            for l in (2, 3):
                w1n_sb[l] = cpool.tile([H, H], bf, tag=f"w1n_{l}", name=f"w1nsb_{l}")
                nc.sync.dma_start(w1n_sb[l][:], w1n[l][:, :])
            for l in (1, 2):
                b2_sb[l] = cpool.tile([H, 1], f32, tag=f"b2_{l}", name=f"b2sb_{l}")
                nc.sync.dma_start(b2_sb[l][:], b2[l][:, :])
            b2row3_sb = cpool.tile([1, H], bf)
            nc.sync.dma_start(b2row3_sb[:], b2row3[:, :])
            wfc1_sb = cpool.tile([H, H // 2], bf)
            nc.sync.dma_start(wfc1_sb[:], wfc1[:, :])
            bfc1_sb = cpool.tile([H // 2, 1], f32)
            nc.sync.dma_start(bfc1_sb[:], bfc1[:, :])
            wfc2_sb = cpool.tile([H // 2, 1], bf)
            nc.sync.dma_start(wfc2_sb[:], wfc2[:, :])
            bfc2_sb = cpool.tile([1, 1], f32)
            nc.sync.dma_start(bfc2_sb[:], bfc2[:, :])

            y_in = {l: dpool.tile([PAD_N, H], bf, tag=f"y_in_{l}", name=f"y_in_{l}")
                    for l in (2, 3)}
            y_g = {l: dpool.tile([nco * PAD_N, H], bf, tag=f"y_g_{l}", name=f"y_g_{l}")
                   for l in (2, 3)}
            # chunked AllGather staging: per chunk, a Shared landing tensor
            NCHK = 4
            chunk_w = [(NW * c // NCHK, NW * (c + 1) // NCHK) for c in range(NCHK)]
            y_ck = {(l, c): dpool.tile(
                [nco * (w1 - w0) * W, H], bf, tag=f"y_ck_{l}_{c}",
                name=f"y_ck_{l}_{c}", addr_space="Shared")
                for l in (2, 3) for c, (w0, w1) in enumerate(chunk_w)}

            pool_tiles = {}

            def run_layer(l, gtab, selftab):
                """One GIN layer: gather+aggregate into [H, W] PSUM windows,
                MLP, and either y_{l+1} stores or the layer-3 pooling tail."""
                # gather tiles per src core, rotated per group
                MAXC = cfg.MAXC
                for g in range(NG):
                    for (gg, s, C0, C1) in gcalls:
                        if gg != g:
                            continue
                        ncal = C1 - C0
                        gt = gpool.tile([P, NCAL_MAX[s], H], bf, tag=f"g{s}",
                                        name=f"g{l}_{g}_{s}")
                        for c0 in range(0, ncal, MAXC):
                            ncc = min(MAXC, ncal - c0)
                            i0 = (slotbase_s[s] + (C0 + c0) * P) // 16
                            i1 = (slotbase_s[s] + (C0 + c0 + ncc) * P) // 16
                            nc.gpsimd.dma_gather(
                                gt[:, c0:c0 + ncc, :],
                                gtab[s * PAD_N:(s + 1) * PAD_N, :],
                                eidx_sb[:, i0:i1], ncc * P, ncc * P, H,
                                queue_num=qctr[0] % 4)
                            qctr[0] += 1
                        gtiles[s] = (gt, C0)
                    for w in range(g * GWIN, min((g + 1) * GWIN, NW)):
                        cols = window_cols[w]
                        ncw = len(cols)
                        # build all S columns of this window in one op
                        S = spool.tile([P, NCW_MAX, W], bf, tag="s", name=f"s{l}_{w}")
                        c0 = int(meta["cw0"][w])
                        in0 = edst_sb[:, c0:c0 + ncw].rearrange(
                            "p (c o) -> p c o", c=ncw).to_broadcast([P, ncw, W])
                        in1 = iota_h[:].rearrange(
                            "p (o q) -> p o q", o=1).to_broadcast([P, ncw, W])
                        nc.vector.tensor_tensor(out=S[:, :ncw, :], in0=in0, in1=in1,
                                                op=mybir.AluOpType.is_equal)
                        zp = zpool.tile([H, W], f32, tag="zp", name=f"zp{l}_{w}")
                        for j, (s, cs) in enumerate(cols):
                            gt, C0 = gtiles[s]
                            nc.tensor.matmul(out=zp[:], lhsT=gt[:, cs - C0, :],
                                             rhs=S[:, j, :], start=(j == 0),
                                             stop=False)
                        sf = sfpool.tile([P, 2, H], bf, tag="sf", name=f"sf{l}_{w}")
                        nc.scalar.dma_start(
                            sf[:], selftab[w * W:(w + 1) * W, :].rearrange(
                                "(c p) h -> p c h", c=2))
                        nc.tensor.matmul(out=zp[:, 0:P], lhsT=sf[:, 0, :], rhs=ident[:],
                                         start=False, stop=False)
                        nc.tensor.matmul(out=zp[:, P:W], lhsT=sf[:, 1, :], rhs=ident[:],
                                         start=False, stop=True)
                        h1 = wpool.tile([H, W], bf, tag="h1", name=f"h1_{l}_{w}")
                        nc.scalar.activation(out=h1[:], in_=zp[:], func=RELU,
                                             bias=bnt_sb[l][:, :1],
                                             scale=bns_sb[l][:, :1])
                        if l < 3:
                            h2p = mmpool.tile([H, W], f32, tag="mm", name=f"h2p{l}_{w}")
                            nc.tensor.matmul(out=h2p[:], lhsT=w2_sb[l][:], rhs=h1[:],
                                             start=True, stop=True)
                            h1f = wpool.tile([H, W], bf, tag="h1f", name=f"h1f{l}_{w}")
                            nc.scalar.activation(out=h1f[:], in_=h2p[:], func=RELU,
                                                 bias=b2_sb[l][:, :1], scale=1.0)
                            ynp = mmpool.tile([P, 2, H], f32, tag="mm",
                                              name=f"ynp{l}_{w}")
                            for i in (0, 1):
                                nc.tensor.matmul(out=ynp[:, i, :],
                                                 lhsT=h1f[:, i * P:(i + 1) * P],
                                                 rhs=w1n_sb[l + 1][:],
                                                 start=True, stop=True)
                            ysb = ypool.tile([P, 2, H], bf, tag="ysb",
                                             name=f"ysb{l}_{w}")
                            nc.vector.tensor_copy(ysb[:], ynp[:])
                            nc.sync.dma_start(
                                y_in[l + 1][w * W:(w + 1) * W, :].rearrange(
                                    "(c p) h -> p c h", c=2), ysb[:])
                            for c, (w0, w1) in enumerate(chunk_w):
                                if w + 1 != w1:
                                    continue
                                r0, r1 = w0 * W, w1 * W
                                ck = y_ck[(l + 1, c)]
                                nc.gpsimd.collective_compute(
                                    "AllGather", mybir.AluOpType.bypass,
                                    replica_groups=[list(range(nco))],
                                    ins=[y_in[l + 1][r0:r1, :].opt()],
                                    outs=[ck[:].opt()])
                                nrc = r1 - r0
                                for s in range(nco):
                                    nc.sync.dma_start(
                                        y_g[l + 1][s * PAD_N + r0:
                                                   s * PAD_N + r1, :],
                                        ck[s * nrc:(s + 1) * nrc, :])
                        else:
                            for i in (0, 1):
                                t = 2 * w + i
                                h3p = mmpool.tile([P, H], f32, tag="mm",
                                                  name=f"h3p{t}")
                                nc.tensor.matmul(out=h3p[:],
                                                 lhsT=h1[:, i * P:(i + 1) * P],
                                                 rhs=w2_sb[3][:], start=True,
                                                 stop=False)
                                nc.tensor.matmul(out=h3p[:], lhsT=ones_row[:],
                                                 rhs=b2row3_sb[:], start=False,
                                                 stop=True)
                                h3t = wpool.tile([P, H], bf, tag="h3t", name=f"h3t{t}")
                                nc.scalar.activation(out=h3t[:], in_=h3p[:], func=RELU)
                                for J in range(JB):
                                    if not (TJ0[J] <= t < TJ1[J]):
                                        continue
                                    sg = spool.tile([P, P], bf, tag="sg",
                                                    name=f"sg{t}_{J}")
                                    nc.vector.scalar_tensor_tensor(
                                        out=sg[:],
                                        in0=batch_sb[:, t:t + 1].to_broadcast([P, P]),
                                        scalar=float(J * P),
                                        op0=mybir.AluOpType.subtract,
                                        in1=iota_h[:, :P],
                                        op1=mybir.AluOpType.is_equal)
                                    if J not in pool_tiles:
                                        pool_tiles[J] = ppool.tile(
                                            [H, P], f32, tag="plp", name=f"plt{J}")
                                    nc.tensor.matmul(
                                        out=pool_tiles[J][:], lhsT=h3t[:], rhs=sg[:],
                                        start=(t == TJ0[J]), stop=(t == TJ1[J] - 1))
                                    if t == TJ1[J] - 1:
                                        pool_sb = wpool.tile([H, P], bf, tag="pool_sb",
                                                             name=f"pool_sb{J}")
                                        nc.vector.tensor_copy(pool_sb[:],
                                                              pool_tiles[J][:])
                                        f1p = mmpool.tile([H // 2, P], f32, tag="mm",
                                                          name=f"f1p{J}")
                                        nc.tensor.matmul(out=f1p[:], lhsT=wfc1_sb[:],
                                                         rhs=pool_sb[:], start=True,
                                                         stop=True)
                                        f1 = wpool.tile([H // 2, P], bf, tag="f1",
                                                        name=f"f1{J}")
                                        nc.scalar.activation(out=f1[:], in_=f1p[:],
                                                             func=RELU,
                                                             bias=bfc1_sb[:, :1],
                                                             scale=1.0)
                                        f2p = mmpool.tile([1, P], f32, tag="mm",
                                                          name=f"f2p{J}")
                                        nc.tensor.matmul(out=f2p[:], lhsT=wfc2_sb[:],
                                                         rhs=f1[:], start=True,
                                                         stop=True)
                                        ojt = ypool.tile([1, P], f32, tag="ojt",
                                                         name=f"ojt{J}")
                                        nc.scalar.activation(out=ojt[:], in_=f2p[:],
                                                             func=IDENT,
                                                             bias=bfc2_sb[:, :1],
                                                             scale=1.0)
                                        nc.sync.dma_start(
                                            out[:1, J * P:(J + 1) * P], ojt[:])

            gtiles = {}
            qctr = [0]
            run_layer(1, y0g, y0l)
            run_layer(2, y_g[2], y_in[2])
            run_layer(3, y_g[3], y_in[3])

    _fix_gather_queues(nc)
    nc.finalize()
    return nc


_WEIGHT_KEYS = tuple(
    f"{p}_{l}" for l in (1, 2, 3)
    for p in ("w1", "b1", "gamma", "beta", "rmean", "rvar", "w2", "b2")
) + ("w_fc1", "b_fc1", "w_fc2", "b_fc2")


def make_in_maps(cfg, meta, percore, x, weights):
    nco = cfg.NCORES
    PAD_N = meta["PAD_N"]
    node_start, counts = meta["node_start"], meta["counts"]
    H = cfg.H

    w = {k: np.asarray(v, np.float32) for k, v in weights.items()}
    y0_full = (x.astype(np.float32) @ w["w1_1"]).astype(NPF)  # [N, H]

    y0g = np.zeros((nco * PAD_N, H), NPF)
    y0ls = []
    for k in range(nco):
        ys = y0_full[node_start[k]:node_start[k + 1]]
        y0g[k * PAD_N:k * PAD_N + counts[k]] = ys
        yl = np.zeros((PAD_N, H), NPF)
        yl[:counts[k]] = ys
        y0ls.append(yl)

    folded = {}
    for l in (1, 2, 3):
        s, t = fold_bn(w[f"b1_{l}"], w[f"gamma_{l}"], w[f"beta_{l}"],
                       w[f"rmean_{l}"], w[f"rvar_{l}"])
        folded[f"bn_s_{l}"] = s.reshape(H, 1)
        folded[f"bn_t_{l}"] = t.reshape(H, 1)

    common = dict(
        y0g=y0g,
        w2_1=w["w2_1"].astype(NPF), w2_2=w["w2_2"].astype(NPF),
        w2_3=w["w2_3"].astype(NPF),
        w1n_2=w["w1_2"].astype(NPF), w1n_3=w["w1_3"].astype(NPF),
        b2_1=w["b2_1"].reshape(H, 1), b2_2=w["b2_2"].reshape(H, 1),
        b2row3=w["b2_3"].reshape(1, H).astype(NPF),
        wfc1=w["w_fc1"].astype(NPF), bfc1=w["b_fc1"].reshape(H // 2, 1),
        wfc2=w["w_fc2"].astype(NPF), bfc2=w["b_fc2"].reshape(1, 1),
        **folded,
    )
    in_maps = []
    for k in range(nco):
        in_maps.append(dict(
            common,
            y0l=y0ls[k],
            eidx=percore["eidx"][k],
            edst=percore["edst"][k],
            batchT=percore["batchT"][k],
        ))
    return in_maps


def assemble_output(cfg, results):
    outs = []
    for k in range(cfg.NCORES):
        outs.append(results[k]["out"][0, :cfg.G_PER_CORE])
    return np.concatenate(outs).reshape(cfg.G, 1).astype(np.float32)


# ============================================================================
# Self-contained kernel entry point
# ============================================================================

N_NODES = 200000
N_EDGES = 600000
N_GRAPHS = 10000
F_NODE = 32
H_DIM = 128
N_CORES = 8

_CACHE = {}


def kernel(**inputs):
    """Full-input GIN GNN forward on 8 TRN2 NeuronCores.

    Takes the unsharded inputs of reference.setup_inputs(), distributes the
    graph across 8 cores internally, and returns the [N_GRAPHS, 1] float32
    output. edge_attr only feeds a dead branch of the reference and is unused.
    """
    x = np.asarray(inputs["x"], np.float32)
    edge_index = np.asarray(inputs["edge_index"])
    batch = np.asarray(inputs["batch"])
    weights = {k: np.asarray(inputs[k], np.float32) for k in _WEIGHT_KEYS}

    cfg = Cfg(N_NODES, N_EDGES, N_GRAPHS, F_NODE, H_DIM, N_CORES)
    key = (edge_index.tobytes(), batch.tobytes())
    ck = _CACHE.get("graph_key")
    if ck != key:
        meta, percore = preprocess_graph(cfg, edge_index, batch)
        nc = build(cfg, meta)
        _CACHE.update(graph_key=key, meta=meta, percore=percore, nc=nc)
    meta, percore, nc = _CACHE["meta"], _CACHE["percore"], _CACHE["nc"]

    in_maps = make_in_maps(cfg, meta, percore, x, weights)
    res = bass_utils.run_bass_kernel_spmd(nc, in_maps, core_ids=list(range(N_CORES)))
    return assemble_output(cfg, res.results)


def run_traced(**inputs):
    """Like kernel() but with NTFF tracing; returns (output, exec_time_ns)."""
    import types as _types

    def _install_hook_shim():
        import antenv
        if "antenv.axon_hooks" in sys.modules:
            return
        try:
            from trn_agent_boot.trn_boot import _ntff_profile_via_ctypes
            hook = _ntff_profile_via_ctypes("/opt/axon/libaxon_pjrt.so")
        except Exception:
            hook = None
        mod = _types.ModuleType("antenv.axon_hooks")
        mod.get_axon_ntff_profile_hook = lambda: hook
        mod.set_axon_ntff_profile_hook = lambda h: None
        sys.modules["antenv.axon_hooks"] = mod
        antenv.axon_hooks = mod

    _install_hook_shim()
    import tempfile
    x = np.asarray(inputs["x"], np.float32)
    edge_index = np.asarray(inputs["edge_index"])
    batch = np.asarray(inputs["batch"])
    weights = {k: np.asarray(inputs[k], np.float32) for k in _WEIGHT_KEYS}
    cfg = Cfg(N_NODES, N_EDGES, N_GRAPHS, F_NODE, H_DIM, N_CORES)
    meta, percore = preprocess_graph(cfg, edge_index, batch)
    nc = build(cfg, meta)
    in_maps = make_in_maps(cfg, meta, percore, x, weights)
    tmpdir = tempfile.mkdtemp(prefix="gnn_ntff_")
    res = bass_utils.run_bass_kernel_spmd(nc, in_maps, core_ids=list(range(N_CORES)),
                                          trace=True, tmpdir=tmpdir)
    return assemble_output(cfg, res.results), res.exec_time_ns


# revision 6
# speedup vs baseline: 1.4910x; 1.4910x over previous
"""GIN GNN kernel for 8 TRN2 NeuronCores — batched-gather S-matmul design (v2).

Key idea vs v1: the v1 kernel issued one gpsimd indirect-DMA per 128 edges
(~1us fixed SWDGE overhead each, 1914 instructions = 2.3ms serialized on
GpSimd). v2 gathers thousands of edge rows per instruction with dma_gather
(994ns + 0.34ns/row), cutting GpSimd busy time ~10x.

Structure (per core, SPMD-identical program, per-core data in tensors):
- Host precomputes y0 = x @ W1_1 (linearity: aggregating y0 == aggregating x
  then multiplying by W1_1), so all 3 layers share one pipeline: gather y rows
  per edge, one-hot S-matmul aggregation into PSUM [H, 256] windows, BN+ReLU
  via scalar activation (per-partition scale/bias), W2 matmul, ReLU, then
  y_{l+1} production via lhsT-swap matmuls (node-major).
- Edges sorted by (dst window of 256 slots, src core, dst slot); per
  (window, src) group padded to 128-edge columns. Per src core, columns are
  gathered from that core's section of the (replicated/AllGathered) y table
  in a few thousand-row dma_gather calls.
- Aggregation: per window w, one 3D is_equal builds all its one-hot S
  columns; one matmul per 128-edge column accumulates into PSUM; self term
  added via identity matmuls from a direct DMA of the window's own y rows.
- Layers 1->2 and 2->3 need an AllGather of the per-core y table (edges cross
  cores since edge_index is random).
- Layer 3 tail: node-major W2 via lhsT swap, pooling via per-graph-block
  one-hot matmuls into PSUM accumulators, FC head per 128-graph block
  (same as v1).
"""

import sys

sys.path.insert(0, "/opt/trn_rl_repo")

import numpy as np
import concourse.bass as bass
import concourse.bacc as bacc
import concourse.bass_isa as bass_isa
import concourse.mybir as mybir
import concourse.tile as tile
from concourse import bass_utils
from concourse.masks import make_identity

P = 128
W = 256           # PSUM aggregation window (dst slots)
BN_EPS = 1e-5
NPF = np.float16  # host dtype matching mybir.dt.float16


class Cfg:
    def __init__(self, n_nodes, n_edges, n_graphs, f_node, h, ncores, gwin=16,
                 maxc=6):
        self.N, self.E, self.G, self.F, self.H = n_nodes, n_edges, n_graphs, f_node, h
        self.NCORES = ncores
        self.G_PER_CORE = n_graphs // ncores
        self.GWIN = gwin  # windows per gather group
        self.MAXC = maxc  # max 128-row columns per dma_gather call (ring cap)


def preprocess_graph(cfg, edge_index, batch):
    """Partition nodes by graph quantiles; organize edges into
    (dst-window, src-core) column groups shared across layers."""
    nco, gpc = cfg.NCORES, cfg.G_PER_CORE
    batch = np.asarray(batch)
    ei = np.asarray(edge_index)
    src, dst = ei[0].astype(np.int64), ei[1].astype(np.int64)

    node_start = np.searchsorted(batch, np.arange(nco + 1) * gpc, side="left")
    counts = np.diff(node_start)
    PAD_N = int(np.ceil(counts.max() / W) * W)
    assert PAD_N < 32768, PAD_N  # int16 gather index range
    NW = PAD_N // W
    NT = PAD_N // P

    core_of = np.searchsorted(node_start, np.arange(cfg.N), side="right") - 1
    local = np.arange(cfg.N) - node_start[core_of]

    dcore = core_of[dst]
    dloc = local[dst]
    dwin = dloc // W
    dslot = dloc % W
    score = core_of[src]
    sloc = local[src]

    # per-core, per-(window, src-core) counts
    cnt = np.zeros((nco, NW, nco), np.int64)
    np.add.at(cnt, (dcore, dwin, score), 1)
    ncols = np.maximum(np.ceil(cnt / P).astype(np.int64).max(axis=0), 1)  # [NW, nco]

    # column layout: per src-core stream (window-major); global edst layout
    # (window-major, src-major inside window)
    colbase_s = np.zeros((nco, NW + 1), np.int64)  # per src: col offset per window
    for s in range(nco):
        colbase_s[s, 1:] = np.cumsum(ncols[:, s])
    total_cols_s = colbase_s[:, -1]              # [nco]
    slotbase_s = np.concatenate([[0], np.cumsum(total_cols_s * P)])
    SLOTS_TOTAL = int(slotbase_s[-1])

    # global edst column index for (w, s, j)
    ncw = ncols.sum(axis=1)                      # cols per window
    cw0 = np.concatenate([[0], np.cumsum(ncw)])  # window col offsets
    TOTC = int(cw0[-1])
    NCW_MAX = int(ncw.max())

    # window_cols[w] = list of (s, col_in_s_stream)
    window_cols = []
    for w in range(NW):
        cols = []
        for s in range(nco):
            for j in range(ncols[w, s]):
                cols.append((s, int(colbase_s[s, w] + j)))
        window_cols.append(cols)

    # gather groups: per src-core, window ranges of GWIN
    GWIN = cfg.GWIN
    NG = int(np.ceil(NW / GWIN))
    gcalls = []  # (s, C0, C1) column range in s's stream
    NCAL_MAX = np.zeros(nco, np.int64)
    for g in range(NG):
        w0, w1 = g * GWIN, min((g + 1) * GWIN, NW)
        for s in range(nco):
            C0, C1 = int(colbase_s[s, w0]), int(colbase_s[s, w1])
            gcalls.append((g, s, C0, C1))
            NCAL_MAX[s] = max(NCAL_MAX[s], C1 - C0)

    # fill per-core eidx / edst
    eidx = np.zeros((nco, SLOTS_TOTAL), np.int16)
    edst = np.full((nco, P, TOTC), 3000.0, np.float16)
    order = np.lexsort((dslot, score, dwin, dcore))
    k_s, w_s, s_s = dcore[order], dwin[order], score[order]
    sl_s, slot_s = sloc[order], dslot[order]
    grp = (k_s * NW + w_s) * nco + s_s
    grp_change = np.concatenate([[True], grp[1:] != grp[:-1]])
    grp_first = np.where(grp_change)[0]
    grp_id = np.cumsum(grp_change) - 1
    pos = np.arange(len(order)) - grp_first[grp_id]   # pos within (k,w,s) group
    col_in_grp = pos // P
    row = pos % P
    slotpos = slotbase_s[s_s] + (colbase_s[s_s, w_s] + col_in_grp) * P + row
    eidx[k_s, slotpos] = sl_s.astype(np.int16)
    gcol = cw0[w_s] + (colbase_s[s_s, w_s] - colbase_s[np.zeros_like(s_s), w_s] * 0)
    # global edst col: cw0[w] + (cols of src cores < s in window w) + col_in_grp
    pre_s = np.zeros((NW, nco), np.int64)
    pre_s[:, 1:] = np.cumsum(ncols[:, :-1], axis=1)
    gcol = cw0[w_s] + pre_s[w_s, s_s] + col_in_grp
    edst[k_s, row, gcol] = slot_s

    # wrap eidx: pos i -> [i%16, i//16], replicate to 128 partitions
    eidx_w = np.empty((nco, 128, SLOTS_TOTAL // 16), np.int16)
    for k in range(nco):
        eidx_w[k] = np.tile(eidx[k].reshape(-1, 16).T, (8, 1))

    # batchT + pooling ranges (natural node order)
    batchT = np.full((nco, P, NT), -1.0, np.float16)
    JB = int(np.ceil(gpc / P))
    TJ0 = np.full(JB, NT, np.int64)
    TJ1 = np.zeros(JB, np.int64)
    for k in range(nco):
        bl = batch[node_start[k]:node_start[k + 1]] - k * gpc
        bt = np.full(PAD_N, -1.0, np.float32)
        bt[:counts[k]] = bl
        batchT[k] = bt.reshape(NT, P).T
        for J in range(JB):
            lo = np.searchsorted(bl, J * P, side="left")
            hi = np.searchsorted(bl, min((J + 1) * P, gpc), side="left")
            if hi > lo:
                TJ0[J] = min(TJ0[J], lo // P)
                TJ1[J] = max(TJ1[J], (hi - 1) // P + 1)
    TJ0 = np.minimum(TJ0, TJ1)

    meta = dict(PAD_N=PAD_N, NW=NW, NT=NT, SLOTS_TOTAL=SLOTS_TOTAL, TOTC=TOTC,
                NCW_MAX=NCW_MAX, ncols=ncols, window_cols=window_cols,
                gcalls=gcalls, NCAL_MAX=NCAL_MAX.tolist(), colbase_s=colbase_s,
                slotbase_s=slotbase_s, cw0=cw0, NG=NG,
                JB=JB, TJ0=TJ0.tolist(), TJ1=TJ1.tolist(),
                node_start=node_start, counts=counts)
    percore = dict(eidx=eidx_w, edst=edst, batchT=batchT)
    return meta, percore


def _fix_gather_queues(nc):
    """Rewrite dma_gather queue_num to match the tile sem-lane rotation.

    Tile assigns the 8 SWDGE semaphore lanes round-robin over Pool-engine DMA
    instructions in final scheduled order; the runtime locks each sem lane to
    one SWDGE queue. Setting queue = lane % 4 post-scheduling keeps the
    mapping consistent regardless of how the scheduler ordered the gathers.
    """
    ctr = 0
    for f in nc.m.functions:
        for bb in f.blocks:
            for ins in bb.instructions:
                if (isinstance(ins, bass_isa.AnyDMAInstruction)
                        and ins.engine == mybir.EngineType.Pool
                        and not isinstance(
                            ins, getattr(bass_isa, "UserSyncedRemoteDMADescs",
                                         ()))):
                    if isinstance(ins, mybir.InstDMAGatherAnt):
                        ins.queue_num = (ctr % 8) % 4
                    ctr += 1


def fold_bn(w1b, gamma, beta, rmean, rvar):
    s = gamma / np.sqrt(rvar + BN_EPS)
    t = (w1b - rmean) * s + beta
    return s.astype(np.float32), t.astype(np.float32)


def build(cfg, meta):
    H = cfg.H
    nco = cfg.NCORES
    PAD_N, NW, NT = meta["PAD_N"], meta["NW"], meta["NT"]
    SLOTS_TOTAL, TOTC, NCW_MAX = meta["SLOTS_TOTAL"], meta["TOTC"], meta["NCW_MAX"]
    window_cols, gcalls = meta["window_cols"], meta["gcalls"]
    colbase_s, slotbase_s = meta["colbase_s"], meta["slotbase_s"]
    NCAL_MAX = meta["NCAL_MAX"]
    NG, GWIN = meta["NG"], cfg.GWIN
    JB, TJ0, TJ1 = meta["JB"], meta["TJ0"], meta["TJ1"]
    f32 = mybir.dt.float32
    bf = mybir.dt.float16

    nc = bacc.Bacc("TRN2", target_bir_lowering=False, debug=False, num_devices=nco,
                   enable_asserts=False, num_swdge_queues=4)
    tc = tile.TileContext(nc, num_cores=nco)

    def dram_in(name, shape, dt=f32):
        return nc.dram_tensor(name, shape, dt, kind="ExternalInput").ap()

    y0g = dram_in("y0g", [nco * PAD_N, H], bf)      # replicated y0 = x@W1_1
    y0l = dram_in("y0l", [PAD_N, H], bf)            # own slice (self term)
    eidx = dram_in("eidx", [P, SLOTS_TOTAL // 16], mybir.dt.int16)
    edst = dram_in("edst", [P, TOTC], bf)
    batchT = dram_in("batchT", [P, NT], bf)
    w2 = {l: dram_in(f"w2_{l}", [H, H], bf) for l in (1, 2, 3)}
    w1n = {l: dram_in(f"w1n_{l}", [H, H], bf) for l in (2, 3)}
    bn_s = {l: dram_in(f"bn_s_{l}", [H, 1]) for l in (1, 2, 3)}
    bn_t = {l: dram_in(f"bn_t_{l}", [H, 1]) for l in (1, 2, 3)}
    b2 = {l: dram_in(f"b2_{l}", [H, 1]) for l in (1, 2)}
    b2row3 = dram_in("b2row3", [1, H], bf)
    wfc1 = dram_in("wfc1", [H, H // 2], bf)
    bfc1 = dram_in("bfc1", [H // 2, 1])
    wfc2 = dram_in("wfc2", [H // 2, 1], bf)
    bfc2 = dram_in("bfc2", [1, 1])

    out = nc.dram_tensor("out", [1, JB * P], f32, kind="ExternalOutput").ap()

    RELU = mybir.ActivationFunctionType.Relu
    IDENT = mybir.ActivationFunctionType.Identity

    with tc:
        with (
            tc.tile_pool(name="const", bufs=1) as cpool,
            tc.tile_pool(name="gat", bufs=2) as gpool,
            tc.tile_pool(name="smat", bufs=3) as spool,
            tc.tile_pool(name="self", bufs=3) as sfpool,
            tc.tile_pool(name="work", bufs=3) as wpool,
            tc.tile_pool(name="yout", bufs=3) as ypool,
            tc.tile_pool(name="zps", bufs=2, space="PSUM") as zpool,
            tc.tile_pool(name="mmps", bufs=3, space="PSUM") as mmpool,
            tc.tile_pool(name="pool_ps", bufs=3, space="PSUM") as ppool,
            tc.tile_pool(name="dram", bufs=1, space="DRAM") as dpool,
        ):
            # ---- constants ----
            iota_i = cpool.tile([P, W], mybir.dt.int32)
            nc.gpsimd.iota(iota_i[:], pattern=[[1, W]], base=0, channel_multiplier=0)
            iota_f = cpool.tile([P, W], f32)
            nc.vector.tensor_copy(iota_f[:], iota_i[:])
            iota_h = cpool.tile([P, W], bf)
            nc.vector.tensor_copy(iota_h[:], iota_i[:])
            ident = cpool.tile([P, P], bf)
            make_identity(nc, ident[:])
            ones_row = cpool.tile([1, P], bf)
            nc.vector.memset(ones_row[:], 1.0)

            eidx_sb = cpool.tile([P, SLOTS_TOTAL // 16], mybir.dt.int16)
            nc.sync.dma_start(eidx_sb[:], eidx[:, :])
            edst_sb = cpool.tile([P, TOTC], bf)
            nc.sync.dma_start(edst_sb[:], edst[:, :])
            batch_sb = cpool.tile([P, NT], bf)
            nc.sync.dma_start(batch_sb[:], batchT[:, :])

            w2_sb, w1n_sb, bns_sb, bnt_sb, b2_sb = {}, {}, {}, {}, {}
            for l in (1, 2, 3):
                w2_sb[l] = cpool.tile([H, H], bf, tag=f"w2_{l}", name=f"w2sb_{l}")
                nc.sync.dma_start(w2_sb[l][:], w2[l][:, :])
                bns_sb[l] = cpool.tile([H, 1], f32, tag=f"bns_{l}", name=f"bnssb_{l}")
                nc.sync.dma_start(bns_sb[l][:], bn_s[l][:, :])
                bnt_sb[l] = cpool.tile([H, 1], f32, tag=f"bnt_{l}", name=f"bntsb_{l}")
                nc.sync.dma_start(bnt_sb[l][:], bn_t[l][:, :])
            for l in (2, 3):
                w1n_sb[l] = cpool.tile([H, H], bf, tag=f"w1n_{l}", name=f"w1nsb_{l}")
                nc.sync.dma_start(w1n_sb[l][:], w1n[l][:, :])
            for l in (1, 2):
                b2_sb[l] = cpool.tile([H, 1], f32, tag=f"b2_{l}", name=f"b2sb_{l}")
                nc.sync.dma_start(b2_sb[l][:], b2[l][:, :])
            b2row3_sb = cpool.tile([1, H], bf)
            nc.sync.dma_start(b2row3_sb[:], b2row3[:, :])
            wfc1_sb = cpool.tile([H, H // 2], bf)
            nc.sync.dma_start(wfc1_sb[:], wfc1[:, :])
            bfc1_sb = cpool.tile([H // 2, 1], f32)
            nc.sync.dma_start(bfc1_sb[:], bfc1[:, :])
            wfc2_sb = cpool.tile([H // 2, 1], bf)
            nc.sync.dma_start(wfc2_sb[:], wfc2[:, :])
            bfc2_sb = cpool.tile([1, 1], f32)
            nc.sync.dma_start(bfc2_sb[:], bfc2[:, :])

            y_in = {l: dpool.tile([PAD_N, H], bf, tag=f"y_in_{l}", name=f"y_in_{l}")
                    for l in (2, 3)}
            y_g = {l: dpool.tile([nco * PAD_N, H], bf, tag=f"y_g_{l}", name=f"y_g_{l}",
                                 addr_space="Shared") for l in (2, 3)}

            pool_tiles = {}

            def run_layer(l, gtab, selftab):
                """One GIN layer: gather+aggregate into [H, W] PSUM windows,
                MLP, and either y_{l+1} stores or the layer-3 pooling tail."""
                # gather tiles per src core, rotated per group
                MAXC = cfg.MAXC
                for g in range(NG):
                    for (gg, s, C0, C1) in gcalls:
                        if gg != g:
                            continue
                        ncal = C1 - C0
                        gt = gpool.tile([P, NCAL_MAX[s], H], bf, tag=f"g{s}",
                                        name=f"g{l}_{g}_{s}")
                        for c0 in range(0, ncal, MAXC):
                            ncc = min(MAXC, ncal - c0)
                            i0 = (slotbase_s[s] + (C0 + c0) * P) // 16
                            i1 = (slotbase_s[s] + (C0 + c0 + ncc) * P) // 16
                            nc.gpsimd.dma_gather(
                                gt[:, c0:c0 + ncc, :],
                                gtab[s * PAD_N:(s + 1) * PAD_N, :],
                                eidx_sb[:, i0:i1], ncc * P, ncc * P, H,
                                queue_num=qctr[0] % 4)
                            qctr[0] += 1
                        gtiles[s] = (gt, C0)
                    for w in range(g * GWIN, min((g + 1) * GWIN, NW)):
                        cols = window_cols[w]
                        ncw = len(cols)
                        # build all S columns of this window in one op
                        S = spool.tile([P, NCW_MAX, W], bf, tag="s", name=f"s{l}_{w}")
                        c0 = int(meta["cw0"][w])
                        in0 = edst_sb[:, c0:c0 + ncw].rearrange(
                            "p (c o) -> p c o", c=ncw).to_broadcast([P, ncw, W])
                        in1 = iota_h[:].rearrange(
                            "p (o q) -> p o q", o=1).to_broadcast([P, ncw, W])
                        nc.vector.tensor_tensor(out=S[:, :ncw, :], in0=in0, in1=in1,
                                                op=mybir.AluOpType.is_equal)
                        zp = zpool.tile([H, W], f32, tag="zp", name=f"zp{l}_{w}")
                        for j, (s, cs) in enumerate(cols):
                            gt, C0 = gtiles[s]
                            nc.tensor.matmul(out=zp[:], lhsT=gt[:, cs - C0, :],
                                             rhs=S[:, j, :], start=(j == 0),
                                             stop=False)
                        sf = sfpool.tile([P, 2, H], bf, tag="sf", name=f"sf{l}_{w}")
                        nc.scalar.dma_start(
                            sf[:], selftab[w * W:(w + 1) * W, :].rearrange(
                                "(c p) h -> p c h", c=2))
                        nc.tensor.matmul(out=zp[:, 0:P], lhsT=sf[:, 0, :], rhs=ident[:],
                                         start=False, stop=False)
                        nc.tensor.matmul(out=zp[:, P:W], lhsT=sf[:, 1, :], rhs=ident[:],
                                         start=False, stop=True)
                        h1 = wpool.tile([H, W], bf, tag="h1", name=f"h1_{l}_{w}")
                        nc.scalar.activation(out=h1[:], in_=zp[:], func=RELU,
                                             bias=bnt_sb[l][:, :1],
                                             scale=bns_sb[l][:, :1])
                        if l < 3:
                            h2p = mmpool.tile([H, W], f32, tag="mm", name=f"h2p{l}_{w}")
                            nc.tensor.matmul(out=h2p[:], lhsT=w2_sb[l][:], rhs=h1[:],
                                             start=True, stop=True)
                            h1f = wpool.tile([H, W], bf, tag="h1f", name=f"h1f{l}_{w}")
                            nc.scalar.activation(out=h1f[:], in_=h2p[:], func=RELU,
                                                 bias=b2_sb[l][:, :1], scale=1.0)
                            ynp = mmpool.tile([P, 2, H], f32, tag="mm",
                                              name=f"ynp{l}_{w}")
                            for i in (0, 1):
                                nc.tensor.matmul(out=ynp[:, i, :],
                                                 lhsT=h1f[:, i * P:(i + 1) * P],
                                                 rhs=w1n_sb[l + 1][:],
                                                 start=True, stop=True)
                            ysb = ypool.tile([P, 2, H], bf, tag="ysb",
                                             name=f"ysb{l}_{w}")
                            nc.vector.tensor_copy(ysb[:], ynp[:])
                            nc.sync.dma_start(
                                y_in[l + 1][w * W:(w + 1) * W, :].rearrange(
                                    "(c p) h -> p c h", c=2), ysb[:])
                        else:
                            for i in (0, 1):
                                t = 2 * w + i
                                h3p = mmpool.tile([P, H], f32, tag="mm",
                                                  name=f"h3p{t}")
                                nc.tensor.matmul(out=h3p[:],
                                                 lhsT=h1[:, i * P:(i + 1) * P],
                                                 rhs=w2_sb[3][:], start=True,
                                                 stop=False)
                                nc.tensor.matmul(out=h3p[:], lhsT=ones_row[:],
                                                 rhs=b2row3_sb[:], start=False,
                                                 stop=True)
                                h3t = wpool.tile([P, H], bf, tag="h3t", name=f"h3t{t}")
                                nc.scalar.activation(out=h3t[:], in_=h3p[:], func=RELU)
                                for J in range(JB):
                                    if not (TJ0[J] <= t < TJ1[J]):
                                        continue
                                    sg = spool.tile([P, P], bf, tag="sg",
                                                    name=f"sg{t}_{J}")
                                    nc.vector.scalar_tensor_tensor(
                                        out=sg[:],
                                        in0=batch_sb[:, t:t + 1].to_broadcast([P, P]),
                                        scalar=float(J * P),
                                        op0=mybir.AluOpType.subtract,
                                        in1=iota_h[:, :P],
                                        op1=mybir.AluOpType.is_equal)
                                    if J not in pool_tiles:
                                        pool_tiles[J] = ppool.tile(
                                            [H, P], f32, tag="plp", name=f"plt{J}")
                                    nc.tensor.matmul(
                                        out=pool_tiles[J][:], lhsT=h3t[:], rhs=sg[:],
                                        start=(t == TJ0[J]), stop=(t == TJ1[J] - 1))
                                    if t == TJ1[J] - 1:
                                        pool_sb = wpool.tile([H, P], bf, tag="pool_sb",
                                                             name=f"pool_sb{J}")
                                        nc.vector.tensor_copy(pool_sb[:],
                                                              pool_tiles[J][:])
                                        f1p = mmpool.tile([H // 2, P], f32, tag="mm",
                                                          name=f"f1p{J}")
                                        nc.tensor.matmul(out=f1p[:], lhsT=wfc1_sb[:],
                                                         rhs=pool_sb[:], start=True,
                                                         stop=True)
                                        f1 = wpool.tile([H // 2, P], bf, tag="f1",
                                                        name=f"f1{J}")
                                        nc.scalar.activation(out=f1[:], in_=f1p[:],
                                                             func=RELU,
                                                             bias=bfc1_sb[:, :1],
                                                             scale=1.0)
                                        f2p = mmpool.tile([1, P], f32, tag="mm",
                                                          name=f"f2p{J}")
                                        nc.tensor.matmul(out=f2p[:], lhsT=wfc2_sb[:],
                                                         rhs=f1[:], start=True,
                                                         stop=True)
                                        ojt = ypool.tile([1, P], f32, tag="ojt",
                                                         name=f"ojt{J}")
                                        nc.scalar.activation(out=ojt[:], in_=f2p[:],
                                                             func=IDENT,
                                                             bias=bfc2_sb[:, :1],
                                                             scale=1.0)
                                        nc.sync.dma_start(
                                            out[:1, J * P:(J + 1) * P], ojt[:])

            gtiles = {}
            qctr = [0]
            run_layer(1, y0g, y0l)
            nc.gpsimd.collective_compute(
                "AllGather", mybir.AluOpType.bypass,
                replica_groups=[list(range(nco))],
                ins=[y_in[2][:].opt()], outs=[y_g[2][:].opt()])
            run_layer(2, y_g[2], y_in[2])
            nc.gpsimd.collective_compute(
                "AllGather", mybir.AluOpType.bypass,
                replica_groups=[list(range(nco))],
                ins=[y_in[3][:].opt()], outs=[y_g[3][:].opt()])
            run_layer(3, y_g[3], y_in[3])

    _fix_gather_queues(nc)
    nc.finalize()
    return nc


_WEIGHT_KEYS = tuple(
    f"{p}_{l}" for l in (1, 2, 3)
    for p in ("w1", "b1", "gamma", "beta", "rmean", "rvar", "w2", "b2")
) + ("w_fc1", "b_fc1", "w_fc2", "b_fc2")


def make_in_maps(cfg, meta, percore, x, weights):
    nco = cfg.NCORES
    PAD_N = meta["PAD_N"]
    node_start, counts = meta["node_start"], meta["counts"]
    H = cfg.H

    w = {k: np.asarray(v, np.float32) for k, v in weights.items()}
    y0_full = (x.astype(np.float32) @ w["w1_1"]).astype(NPF)  # [N, H]

    y0g = np.zeros((nco * PAD_N, H), NPF)
    y0ls = []
    for k in range(nco):
        ys = y0_full[node_start[k]:node_start[k + 1]]
        y0g[k * PAD_N:k * PAD_N + counts[k]] = ys
        yl = np.zeros((PAD_N, H), NPF)
        yl[:counts[k]] = ys
        y0ls.append(yl)

    folded = {}
    for l in (1, 2, 3):
        s, t = fold_bn(w[f"b1_{l}"], w[f"gamma_{l}"], w[f"beta_{l}"],
                       w[f"rmean_{l}"], w[f"rvar_{l}"])
        folded[f"bn_s_{l}"] = s.reshape(H, 1)
        folded[f"bn_t_{l}"] = t.reshape(H, 1)

    common = dict(
        y0g=y0g,
        w2_1=w["w2_1"].astype(NPF), w2_2=w["w2_2"].astype(NPF),
        w2_3=w["w2_3"].astype(NPF),
        w1n_2=w["w1_2"].astype(NPF), w1n_3=w["w1_3"].astype(NPF),
        b2_1=w["b2_1"].reshape(H, 1), b2_2=w["b2_2"].reshape(H, 1),
        b2row3=w["b2_3"].reshape(1, H).astype(NPF),
        wfc1=w["w_fc1"].astype(NPF), bfc1=w["b_fc1"].reshape(H // 2, 1),
        wfc2=w["w_fc2"].astype(NPF), bfc2=w["b_fc2"].reshape(1, 1),
        **folded,
    )
    in_maps = []
    for k in range(nco):
        in_maps.append(dict(
            common,
            y0l=y0ls[k],
            eidx=percore["eidx"][k],
            edst=percore["edst"][k],
            batchT=percore["batchT"][k],
        ))
    return in_maps


def assemble_output(cfg, results):
    outs = []
    for k in range(cfg.NCORES):
        outs.append(results[k]["out"][0, :cfg.G_PER_CORE])
    return np.concatenate(outs).reshape(cfg.G, 1).astype(np.float32)


# ============================================================================
# Self-contained kernel entry point
# ============================================================================

N_NODES = 200000
N_EDGES = 600000
N_GRAPHS = 10000
F_NODE = 32
H_DIM = 128
N_CORES = 8

_CACHE = {}


def kernel(**inputs):
    """Full-input GIN GNN forward on 8 TRN2 NeuronCores.

    Takes the unsharded inputs of reference.setup_inputs(), distributes the
    graph across 8 cores internally, and returns the [N_GRAPHS, 1] float32
    output. edge_attr only feeds a dead branch of the reference and is unused.
    """
    x = np.asarray(inputs["x"], np.float32)
    edge_index = np.asarray(inputs["edge_index"])
    batch = np.asarray(inputs["batch"])
    weights = {k: np.asarray(inputs[k], np.float32) for k in _WEIGHT_KEYS}

    cfg = Cfg(N_NODES, N_EDGES, N_GRAPHS, F_NODE, H_DIM, N_CORES)
    key = (edge_index.tobytes(), batch.tobytes())
    ck = _CACHE.get("graph_key")
    if ck != key:
        meta, percore = preprocess_graph(cfg, edge_index, batch)
        nc = build(cfg, meta)
        _CACHE.update(graph_key=key, meta=meta, percore=percore, nc=nc)
    meta, percore, nc = _CACHE["meta"], _CACHE["percore"], _CACHE["nc"]

    in_maps = make_in_maps(cfg, meta, percore, x, weights)
    res = bass_utils.run_bass_kernel_spmd(nc, in_maps, core_ids=list(range(N_CORES)))
    return assemble_output(cfg, res.results)


def run_traced(**inputs):
    """Like kernel() but with NTFF tracing; returns (output, exec_time_ns)."""
    import types as _types

    def _install_hook_shim():
        import antenv
        if "antenv.axon_hooks" in sys.modules:
            return
        try:
            from trn_agent_boot.trn_boot import _ntff_profile_via_ctypes
            hook = _ntff_profile_via_ctypes("/opt/axon/libaxon_pjrt.so")
        except Exception:
            hook = None
        mod = _types.ModuleType("antenv.axon_hooks")
        mod.get_axon_ntff_profile_hook = lambda: hook
        mod.set_axon_ntff_profile_hook = lambda h: None
        sys.modules["antenv.axon_hooks"] = mod
        antenv.axon_hooks = mod

    _install_hook_shim()
    import tempfile
    x = np.asarray(inputs["x"], np.float32)
    edge_index = np.asarray(inputs["edge_index"])
    batch = np.asarray(inputs["batch"])
    weights = {k: np.asarray(inputs[k], np.float32) for k in _WEIGHT_KEYS}
    cfg = Cfg(N_NODES, N_EDGES, N_GRAPHS, F_NODE, H_DIM, N_CORES)
    meta, percore = preprocess_graph(cfg, edge_index, batch)
    nc = build(cfg, meta)
    in_maps = make_in_maps(cfg, meta, percore, x, weights)
    tmpdir = tempfile.mkdtemp(prefix="gnn_ntff_")
    res = bass_utils.run_bass_kernel_spmd(nc, in_maps, core_ids=list(range(N_CORES)),
                                          trace=True, tmpdir=tmpdir)
    return assemble_output(cfg, res.results), res.exec_time_ns
